# revision 29
# baseline (speedup 1.0000x reference)
"""Trainium2 Bass kernel for nn_CE_55937654063537.

Reference computation:
    b1 = conv3x3(x, g_w) + g_b            [B, 2, 512, 512]
    b2 = conv1x1(x, theta_w) + theta_b    [B, 2, 512, 512]
    m  = patch_mean(b1, 7) + patch_mean(b2, 7)   [B, 2, 7, 7]
    out = bilinear_upsample(m, 512, 512)  (half-pixel centers)

Everything is linear, so the kernel never materializes the conv outputs.
patch_mean(feat)[i, j] is (1/(H*W)) * the sum of feat over a rectangle that is
the full map minus <=3 boundary rows/cols.  Those rectangle sums are linear in
(a) the column-sum over h of x and (b) 8 boundary rows of x.  Per batch b the
device pipeline is:
  load:    x[b] streamed in 1 MB tiles, 4 consecutive rows per SBUF partition
           (8 KB contiguous DMA runs); boundary rows re-fetched by one tiny DMA
  phase 1: gpsimd pre-adds the 4 row-groups, PE does the 128-partition column
           sum (ones vector, float32r single-pass matmul)
  phase 2: stats reduced to per-row [total + 8 edge columns] (batched DVE ops),
           then tiny fp32 matmuls apply the conv-derived coefficients giving
           R[co, i] summaries [total + 6 edge values]
  phase 3: PE transpose + one small matmul against L -> m^T [j, (co, i)]
  phase 4: per (b, co): out = A @ m @ A^T via float32r matmuls against the
           512x7 bilinear matrix A (strided row slices keep the 4-rows-per-
           partition output layout), then 1 MB DMA stores.
Batches pipeline: batch b+1's loads overlap batch b's output stores.

Data parallel over batch: 8 cores x 4 batches each; params replicated.
"""
import numpy as np

H = W = 512
K = 7
CIN = 4
CO = 2
BLOC = 4    # batches per core
NCORES = 8

_PROG = None          # cached Bass program (weight-independent; weights are inputs)
TRACE = False         # set True (e.g. from test.py) to profile; see LAST_EXEC_NS
LAST_EXEC_NS = None
LAST_TRACE_PATH = None


# ---------------------------------------------------------------------------
# host-side constant builders (all tiny, derived from conv weights)
# ---------------------------------------------------------------------------

def resize_mat(in_size, out_size):
    """Bilinear (half-pixel, edge-normalized) interpolation matrix [out, in],
    matching jax.image.resize(method='bilinear') for upsampling."""
    inv_scale = in_size / out_size
    sample_f = (np.arange(out_size) + 0.5) * inv_scale - 0.5
    xw = np.abs(sample_f[None, :] - np.arange(in_size)[:, None])
    weights = np.maximum(0, 1 - xw)
    total = weights.sum(axis=0, keepdims=True)
    return (weights / total).T.astype(np.float32)  # [out, in]


def build_lhsTR(g_w, g_b, theta_w, theta_b):
    """Phase-2 weight blocks (per batch; identical for every b).

    Returns (blk [4, 3, 9, 14], bias [1, 14]):
      blk[ci, dw, q, col]: coefficient of stats row q of channel ci
        (q: 0=colsum over h, 1..4=x rows 0..3, 5..8=x rows 508..511)
        in output row col = co*7 + i -> R[co, i][w] under w-shift dw.
      bias[0, col]: additive constant (applies to every w of R[col]).
    """
    gw = g_w.astype(np.float64)
    gb = g_b.astype(np.float64)
    tw = theta_w.astype(np.float64)[:, :, 0, 0]
    tb = theta_b.astype(np.float64)
    blk = np.zeros((CIN, 3, 9, 14), dtype=np.float64)
    bias = np.zeros((1, 14), dtype=np.float64)

    def add_F(col, co, dw, sign):
        for ci in range(CIN):
            blk[ci, dw, 0, col] += sign * gw[co, ci, :, dw].sum()
            blk[ci, dw, 1, col] += -sign * gw[co, ci, 2, dw]   # x row 0
            blk[ci, dw, 8, col] += -sign * gw[co, ci, 0, dw]   # x row 511
            if dw == 1:
                blk[ci, dw, 0, col] += sign * tw[co, ci]
        if dw == 1:
            bias[0, col] += sign * H * (gb[co] + tb[co])

    def add_bd(col, co, r, dw, sign):
        for ci in range(CIN):
            for dh in range(3):
                hr = r + dh - 1
                if 0 <= hr < H:
                    q = 1 + hr if hr <= 3 else 5 + (hr - (H - 4))
                    blk[ci, dw, q, col] += sign * gw[co, ci, dh, dw]
            if dw == 1:
                q = 1 + r if r <= 3 else 5 + (r - (H - 4))
                blk[ci, dw, q, col] += sign * tw[co, ci]
        if dw == 1:
            bias[0, col] += sign * (gb[co] + tb[co])

    for co in range(CO):
        for i in range(K):
            col = co * 7 + i
            for dw in range(3):
                add_F(col, co, dw, 1.0)
                if i < 3:
                    for r in range(H - 3 + i, H):
                        add_bd(col, co, r, dw, -1.0)
                elif i > 3:
                    for r in range(0, i - 3):
                        add_bd(col, co, r, dw, -1.0)
    return blk.astype(np.float32), bias.astype(np.float32)


def build_L():
    """Phase-3 lhsT [7, 7] (includes the 1/(H*W) patch-mean scale).

    Row e' order matches the R-summary columns: 0 -> total sum,
    1..3 -> R[w=0..2], 4..6 -> R[w=509..511].
    Column j yields m[i, j] = T_R - partial edge sums."""
    L = np.zeros((7, 7), dtype=np.float64)
    L[0, :] = 1.0
    for j in range(3):            # j=0,1,2: subtract tail elements w >= 509+j
        for e in range(3 + j, 6):
            L[1 + e, j] = -1.0    # e=3,4,5 -> rows 4..6
    for j in range(4, 7):         # j=4,5,6: subtract head elements w < j-3
        for e in range(0, j - 3):
            L[1 + e, j] = -1.0    # e=0,1,2 -> rows 1..3
    return (L / (H * W)).astype(np.float32)


def build_consts(g_w, g_b, theta_w, theta_b):
    blk, biasrow = build_lhsTR(g_w, g_b, theta_w, theta_b)
    A = resize_mat(K, H)          # [512, 7]
    biaspat = np.ones((1, 7), dtype=np.float32)
    biaspat[0, 0] = float(W)      # total-sum column gets bias once per w
    return {
        "blk": blk,
        "biasrow": biasrow,
        "biaspat": biaspat,
        "ident14": np.eye(14, dtype=np.float32),
        "lmat": build_L(),
        "at": np.ascontiguousarray(A.T),                              # [7, 512]
        "atr": np.ascontiguousarray(
            A.reshape(128, 4, K).transpose(1, 2, 0)),                 # [4, 7, 128]
        "ones128": np.ones((128, 1), dtype=np.float32),
    }


# ---------------------------------------------------------------------------
# device program
# ---------------------------------------------------------------------------

def build_program():
    import concourse.bass as bass
    import concourse.bacc as bacc
    import concourse.tile as tile
    from concourse import mybir

    f32 = mybir.dt.float32
    f32r = mybir.dt.float32r
    nc = bacc.Bacc(None, target_bir_lowering=False, enable_partition_id=False)

    xs = nc.dram_tensor("xs", [BLOC, CIN, H, W], f32r, kind="ExternalInput")
    blk_d = nc.dram_tensor("blk", [CIN, 3, 9, 14], f32, kind="ExternalInput")
    bias_d = nc.dram_tensor("biasrow", [1, 14], f32, kind="ExternalInput")
    bpat_d = nc.dram_tensor("biaspat", [1, 7], f32, kind="ExternalInput")
    ident_d = nc.dram_tensor("ident14", [14, 14], f32, kind="ExternalInput")
    lmat_d = nc.dram_tensor("lmat", [7, 7], f32, kind="ExternalInput")
    at_d = nc.dram_tensor("at", [7, 512], f32r, kind="ExternalInput")
    atr_d = nc.dram_tensor("atr", [4, 7, 128], f32r, kind="ExternalInput")
    ones_d = nc.dram_tensor("ones128", [128, 1], f32r, kind="ExternalInput")
    y = nc.dram_tensor("y", [BLOC, CO, H, W], f32, kind="ExternalOutput")

    with tile.TileContext(nc) as tc:
        with (
            tc.tile_pool(name="consts", bufs=1) as consts,
            tc.tile_pool(name="xpool", bufs=8) as xpool,
            tc.tile_pool(name="spool", bufs=2) as spool,
            tc.tile_pool(name="vpool", bufs=2) as vpool,
            tc.tile_pool(name="small", bufs=2) as small,
            tc.tile_pool(name="mtp", bufs=1) as mtp,
            tc.tile_pool(name="tgpool", bufs=2) as tgpool,
            tc.tile_pool(name="obuf", bufs=2) as obuf,
            tc.tile_pool(name="pstats", bufs=2, space="PSUM") as pstats,
            tc.tile_pool(name="pr", bufs=1, space="PSUM") as pr,
            tc.tile_pool(name="pet", bufs=1, space="PSUM") as pet,
            tc.tile_pool(name="pmt", bufs=1, space="PSUM") as pmt,
            tc.tile_pool(name="ptg", bufs=1, space="PSUM") as ptg,
            tc.tile_pool(name="poc", bufs=2, space="PSUM") as poc,
        ):
            c_ones = consts.tile([128, 1], f32r)
            nc.sync.dma_start(out=c_ones, in_=ones_d[:, :])
            c_blk = consts.tile([9, 12, 14], f32)
            nc.sync.dma_start(out=c_blk, in_=blk_d.rearrange("c d q m -> q (c d) m"))
            c_bias = consts.tile([1, 14], f32)
            nc.sync.dma_start(out=c_bias, in_=bias_d[:, :])
            c_bpat = consts.tile([1, 7], f32)
            nc.sync.dma_start(out=c_bpat, in_=bpat_d[:, :])
            c_ident = consts.tile([14, 14], f32)
            nc.sync.dma_start(out=c_ident, in_=ident_d[:, :])
            c_lmat = consts.tile([7, 7], f32)
            nc.sync.dma_start(out=c_lmat, in_=lmat_d[:, :])
            c_at = consts.tile([7, 512], f32r)
            nc.sync.dma_start(out=c_at, in_=at_d[:, :])
            c_atr = consts.tile([7, 4, 128], f32r)
            nc.sync.dma_start(out=c_atr, in_=atr_d.rearrange("t j p -> j t p"))

            Et_ps = pet.tile([7, 56], f32, tag="Et_ps")
            mT_ps = pmt.tile([7, 56], f32, tag="mT_ps")
            mT = mtp.tile([7, 56], f32r, tag="mT")

            def load_phase(b):
                # ---- load + phase 1: column sums ----
                S = spool.tile([9, CIN, 512], f32r, tag="S")
                sts = []
                for ci in range(CIN):
                    xt = xpool.tile([128, 4, 512], f32r, tag="xt")
                    eng = nc.sync if ci % 2 == 0 else nc.scalar
                    eng.dma_start(
                        out=xt,
                        in_=xs[b, ci].rearrange("(p t) w -> p t w", t=4),
                    )
                    st = pstats.tile([1, 512], f32, tag="st")
                    for t in range(4):
                        nc.tensor.matmul(st, c_ones, xt[:, t, :],
                                         start=(t == 0), stop=(t == 3))
                    sts.append(st)
                # boundary rows 0..3 and 508..511 of all 4 channels
                nc.scalar.dma_start(
                    out=S[1:5, :, :],
                    in_=xs[b, :, 0:4, :].rearrange("c r w -> r c w"),
                )
                nc.scalar.dma_start(
                    out=S[5:9, :, :],
                    in_=xs[b, :, 508:512, :].rearrange("c r w -> r c w"),
                )
                for ci in range(CIN):
                    nc.vector.tensor_copy(S[0:1, ci, :], sts[ci])
                return S

            def tail_phase(b, S):
                # ---- phase 2a: per-row summaries V = [T | edges], batched ----
                # V column groups, one per w-shift dw (7 cols each):
                #  dw=0: [T-S511, 0,  S0, S1, S508, S509, S510]
                #  dw=1: [T,      S0, S1, S2, S509, S510, S511]
                #  dw=2: [T-S0,   S1, S2, S3, S510, S511, 0   ]
                V = vpool.tile([9, CIN, 21], f32, tag="V")
                nc.vector.reduce_sum(V[:, :, 7:8], S, axis=mybir.AxisListType.X)
                edges = bass.AP(           # S columns {0,1,2, 509,510,511}
                    tensor=S.tensor, offset=S.offset,
                    ap=[S.ap[0], S.ap[1], [509, 2], [1, 3]],
                )
                nc.vector.tensor_copy(
                    V[:, :, 8:14].rearrange("q c (g e) -> q c g e", g=2), edges)
                nc.vector.tensor_sub(V[:, :, 0:1], V[:, :, 7:8], V[:, :, 13:14])
                nc.gpsimd.memset(V[:, :, 1:2], 0.0)
                nc.gpsimd.tensor_copy(V[:, :, 2:4], V[:, :, 8:10])
                nc.vector.tensor_copy(V[:, :, 4:7], S[:, :, 508:511])
                nc.vector.tensor_sub(V[:, :, 14:15], V[:, :, 7:8], V[:, :, 8:9])
                nc.vector.tensor_copy(V[:, :, 15:18], S[:, :, 1:4])
                nc.gpsimd.tensor_copy(V[:, :, 18:20], V[:, :, 12:14])
                nc.gpsimd.memset(V[:, :, 20:21], 0.0)

                # ---- phase 2b: R summaries [14, 7] via tiny fp32 matmuls ----
                Rb = pr.tile([14, 7], f32, tag="R")
                nc.tensor.matmul(Rb, c_bias, c_bpat, start=True, stop=False)
                for ci in range(CIN):
                    for dw in range(3):
                        last = (ci == CIN - 1 and dw == 2)
                        nc.tensor.matmul(
                            Rb, c_blk[:, ci * 3 + dw, :],
                            V[:, ci, 7 * dw:7 * dw + 7],
                            start=False, stop=last)

                # ---- phase 3: transpose + L -> m^T columns for this b ----
                Ep = small.tile([14, 7], f32, tag="Ep")
                nc.vector.tensor_copy(Ep, Rb)
                nc.tensor.transpose(Et_ps[:, 14 * b:14 * b + 14], Ep, c_ident)
                Etb = small.tile([7, 14], f32, tag="Etb")
                nc.vector.tensor_copy(Etb, Et_ps[:, 14 * b:14 * b + 14])
                nc.tensor.matmul(mT_ps[:, 14 * b:14 * b + 14], c_lmat, Etb,
                                 start=True, stop=True)
                nc.vector.tensor_copy(mT[:, 14 * b:14 * b + 14],
                                      mT_ps[:, 14 * b:14 * b + 14])

                # ---- phase 4: upsample out = A @ m @ A^T, store ----
                for co in range(CO):
                    g = b * CO + co
                    tg_ps = ptg.tile([7, 512], f32, tag="tg_ps")
                    nc.tensor.matmul(tg_ps, mT[:, g * 7:(g + 1) * 7], c_at,
                                     start=True, stop=True)
                    tg = tgpool.tile([7, 512], f32r, tag="tg")
                    nc.vector.tensor_copy(tg, tg_ps)
                    ob = obuf.tile([128, 4, 512], f32, tag="ob")
                    for t in range(4):
                        oc_ps = poc.tile([128, 512], f32, tag="oc")
                        nc.tensor.matmul(oc_ps, c_atr[:, t, :], tg,
                                         start=True, stop=True)
                        if t % 2 == 0:
                            nc.vector.tensor_copy(ob[:, t, :], oc_ps)
                        else:
                            nc.scalar.copy(ob[:, t, :], oc_ps)
                    oeng = nc.sync if co == 0 else nc.scalar
                    oeng.dma_start(
                        out=y[b, co].rearrange("(p t) w -> p t w", t=4),
                        in_=ob,
                    )

            # Software-pipelined emission: batch b's middle/output phases are
            # emitted after batch b+1's loads + column sums, so the in-order
            # PE stream never stalls on b's (DVE-produced) V summaries while
            # b+1's streaming work is ready.
            S_prev = load_phase(0)
            for b in range(1, BLOC):
                S_cur = load_phase(b)
                tail_phase(b - 1, S_prev)
                S_prev = S_cur
            tail_phase(BLOC - 1, S_prev)
    return nc


def _get_prog():
    global _PROG
    if _PROG is None:
        _PROG = build_program()
        _PROG.finalize()
    return _PROG


# ---------------------------------------------------------------------------
# host entry point
# ---------------------------------------------------------------------------

def kernel(x, g_w, g_b, theta_w, theta_b):
    global LAST_EXEC_NS, LAST_TRACE_PATH
    from concourse.bass_utils import run_bass_kernel_spmd

    x = np.ascontiguousarray(np.asarray(x, dtype=np.float32))
    g_w = np.asarray(g_w, dtype=np.float32)
    g_b = np.asarray(g_b, dtype=np.float32)
    theta_w = np.asarray(theta_w, dtype=np.float32)
    theta_b = np.asarray(theta_b, dtype=np.float32)

    consts = build_consts(g_w, g_b, theta_w, theta_b)
    nc = _get_prog()
    in_maps = [
        {"xs": np.ascontiguousarray(x[c * BLOC:(c + 1) * BLOC]), **consts}
        for c in range(NCORES)
    ]
    res = run_bass_kernel_spmd(nc, in_maps, core_ids=list(range(NCORES)),
                               trace=TRACE)
    LAST_EXEC_NS = res.exec_time_ns
    if TRACE and res.instructions_and_trace is not None:
        LAST_TRACE_PATH = res.instructions_and_trace[1]
    return np.concatenate([res.results[c]["y"] for c in range(NCORES)], axis=0)


# revision 30
# speedup vs baseline: 1.1169x; 1.1169x over previous
"""Trainium2 Bass kernel for nn_CE_55937654063537.

Reference computation:
    b1 = conv3x3(x, g_w) + g_b            [B, 2, 512, 512]
    b2 = conv1x1(x, theta_w) + theta_b    [B, 2, 512, 512]
    m  = patch_mean(b1, 7) + patch_mean(b2, 7)   [B, 2, 7, 7]
    out = bilinear_upsample(m, 512, 512)  (half-pixel centers)

Everything is linear, so the kernel never materializes the conv outputs.
patch_mean(feat)[i, j] is (1/(H*W)) * the sum of feat over a rectangle that is
the full map minus <=3 boundary rows/cols.  Those rectangle sums are linear in
(a) the column-sum over h of x and (b) 8 boundary rows of x.  Per batch b the
device pipeline is:
  load:    x[b] streamed in 1 MB tiles, 4 consecutive rows per SBUF partition
           (8 KB contiguous DMA runs); boundary rows re-fetched by one tiny DMA
  phase 1: gpsimd pre-adds the 4 row-groups, PE does the 128-partition column
           sum (ones vector, float32r single-pass matmul)
  phase 2: stats reduced to per-row [total + 8 edge columns] (batched DVE ops),
           then tiny fp32 matmuls apply the conv-derived coefficients giving
           R[co, i] summaries [total + 6 edge values]
  phase 3: PE transpose + one small matmul against L -> m^T [j, (co, i)]
  phase 4: per (b, co): out = A @ m @ A^T via float32r matmuls against the
           512x7 bilinear matrix A (strided row slices keep the 4-rows-per-
           partition output layout), then 1 MB DMA stores.
Batches pipeline: batch b+1's loads overlap batch b's output stores.

Data parallel over batch: 8 cores x 4 batches each; params replicated.
"""
import numpy as np

H = W = 512
K = 7
CIN = 4
CO = 2
BLOC = 4    # batches per core
NCORES = 8

_PROG = None          # cached Bass program (weight-independent; weights are inputs)
TRACE = False         # set True (e.g. from test.py) to profile; see LAST_EXEC_NS
LAST_EXEC_NS = None
LAST_TRACE_PATH = None


# ---------------------------------------------------------------------------
# host-side constant builders (all tiny, derived from conv weights)
# ---------------------------------------------------------------------------

def resize_mat(in_size, out_size):
    """Bilinear (half-pixel, edge-normalized) interpolation matrix [out, in],
    matching jax.image.resize(method='bilinear') for upsampling."""
    inv_scale = in_size / out_size
    sample_f = (np.arange(out_size) + 0.5) * inv_scale - 0.5
    xw = np.abs(sample_f[None, :] - np.arange(in_size)[:, None])
    weights = np.maximum(0, 1 - xw)
    total = weights.sum(axis=0, keepdims=True)
    return (weights / total).T.astype(np.float32)  # [out, in]


def build_lhsTR(g_w, g_b, theta_w, theta_b):
    """Phase-2 weight blocks (per batch; identical for every b).

    Returns (blk [4, 3, 9, 14], bias [1, 14]):
      blk[ci, dw, q, col]: coefficient of stats row q of channel ci
        (q: 0=colsum over h, 1..4=x rows 0..3, 5..8=x rows 508..511)
        in output row col = co*7 + i -> R[co, i][w] under w-shift dw.
      bias[0, col]: additive constant (applies to every w of R[col]).
    """
    gw = g_w.astype(np.float64)
    gb = g_b.astype(np.float64)
    tw = theta_w.astype(np.float64)[:, :, 0, 0]
    tb = theta_b.astype(np.float64)
    blk = np.zeros((CIN, 3, 9, 14), dtype=np.float64)
    bias = np.zeros((1, 14), dtype=np.float64)

    def add_F(col, co, dw, sign):
        for ci in range(CIN):
            blk[ci, dw, 0, col] += sign * gw[co, ci, :, dw].sum()
            blk[ci, dw, 1, col] += -sign * gw[co, ci, 2, dw]   # x row 0
            blk[ci, dw, 8, col] += -sign * gw[co, ci, 0, dw]   # x row 511
            if dw == 1:
                blk[ci, dw, 0, col] += sign * tw[co, ci]
        if dw == 1:
            bias[0, col] += sign * H * (gb[co] + tb[co])

    def add_bd(col, co, r, dw, sign):
        for ci in range(CIN):
            for dh in range(3):
                hr = r + dh - 1
                if 0 <= hr < H:
                    q = 1 + hr if hr <= 3 else 5 + (hr - (H - 4))
                    blk[ci, dw, q, col] += sign * gw[co, ci, dh, dw]
            if dw == 1:
                q = 1 + r if r <= 3 else 5 + (r - (H - 4))
                blk[ci, dw, q, col] += sign * tw[co, ci]
        if dw == 1:
            bias[0, col] += sign * (gb[co] + tb[co])

    for co in range(CO):
        for i in range(K):
            col = co * 7 + i
            for dw in range(3):
                add_F(col, co, dw, 1.0)
                if i < 3:
                    for r in range(H - 3 + i, H):
                        add_bd(col, co, r, dw, -1.0)
                elif i > 3:
                    for r in range(0, i - 3):
                        add_bd(col, co, r, dw, -1.0)
    return blk.astype(np.float32), bias.astype(np.float32)


def build_L():
    """Phase-3 lhsT [7, 7] (includes the 1/(H*W) patch-mean scale).

    Row e' order matches the R-summary columns: 0 -> total sum,
    1..3 -> R[w=0..2], 4..6 -> R[w=509..511].
    Column j yields m[i, j] = T_R - partial edge sums."""
    L = np.zeros((7, 7), dtype=np.float64)
    L[0, :] = 1.0
    for j in range(3):            # j=0,1,2: subtract tail elements w >= 509+j
        for e in range(3 + j, 6):
            L[1 + e, j] = -1.0    # e=3,4,5 -> rows 4..6
    for j in range(4, 7):         # j=4,5,6: subtract head elements w < j-3
        for e in range(0, j - 3):
            L[1 + e, j] = -1.0    # e=0,1,2 -> rows 1..3
    return (L / (H * W)).astype(np.float32)


def build_consts(g_w, g_b, theta_w, theta_b):
    blk, biasrow = build_lhsTR(g_w, g_b, theta_w, theta_b)
    A = resize_mat(K, H)          # [512, 7]
    biaspat = np.ones((1, 7), dtype=np.float32)
    biaspat[0, 0] = float(W)      # total-sum column gets bias once per w
    return {
        "blk": blk,
        "biasrow": biasrow,
        "biaspat": biaspat,
        "ident14": np.eye(14, dtype=np.float32),
        "lmat": build_L(),
        "at": np.ascontiguousarray(A.T),                              # [7, 512]
        "atr": np.ascontiguousarray(
            A.reshape(128, 4, K).transpose(1, 2, 0)),                 # [4, 7, 128]
        "ones128": np.ones((128, 1), dtype=np.float32),
    }


# ---------------------------------------------------------------------------
# device program
# ---------------------------------------------------------------------------

def build_program():
    import concourse.bass as bass
    import concourse.bacc as bacc
    import concourse.tile as tile
    from concourse import mybir

    f32 = mybir.dt.float32
    f32r = mybir.dt.float32r
    nc = bacc.Bacc(None, target_bir_lowering=False, enable_partition_id=False)

    xs = nc.dram_tensor("xs", [BLOC, CIN, H, W], f32r, kind="ExternalInput")
    blk_d = nc.dram_tensor("blk", [CIN, 3, 9, 14], f32, kind="ExternalInput")
    bias_d = nc.dram_tensor("biasrow", [1, 14], f32, kind="ExternalInput")
    bpat_d = nc.dram_tensor("biaspat", [1, 7], f32, kind="ExternalInput")
    ident_d = nc.dram_tensor("ident14", [14, 14], f32, kind="ExternalInput")
    lmat_d = nc.dram_tensor("lmat", [7, 7], f32, kind="ExternalInput")
    at_d = nc.dram_tensor("at", [7, 512], f32r, kind="ExternalInput")
    atr_d = nc.dram_tensor("atr", [4, 7, 128], f32r, kind="ExternalInput")
    ones_d = nc.dram_tensor("ones128", [128, 1], f32r, kind="ExternalInput")
    y = nc.dram_tensor("y", [BLOC, CO, H, W], f32, kind="ExternalOutput")

    with tile.TileContext(nc) as tc:
        with (
            tc.tile_pool(name="consts", bufs=1) as consts,
            tc.tile_pool(name="xpool", bufs=8) as xpool,
            tc.tile_pool(name="spool", bufs=2) as spool,
            tc.tile_pool(name="vpool", bufs=2) as vpool,
            tc.tile_pool(name="small", bufs=2) as small,
            tc.tile_pool(name="mtp", bufs=1) as mtp,
            tc.tile_pool(name="tgpool", bufs=2) as tgpool,
            tc.tile_pool(name="obuf", bufs=2) as obuf,
            tc.tile_pool(name="pstats", bufs=2, space="PSUM") as pstats,
            tc.tile_pool(name="pr", bufs=1, space="PSUM") as pr,
            tc.tile_pool(name="pet", bufs=1, space="PSUM") as pet,
            tc.tile_pool(name="pmt", bufs=1, space="PSUM") as pmt,
            tc.tile_pool(name="ptg", bufs=1, space="PSUM") as ptg,
            tc.tile_pool(name="poc", bufs=2, space="PSUM") as poc,
        ):
            c_ones = consts.tile([128, 1], f32r)
            nc.sync.dma_start(out=c_ones, in_=ones_d[:, :])
            c_blk = consts.tile([9, 12, 14], f32)
            nc.sync.dma_start(out=c_blk, in_=blk_d.rearrange("c d q m -> q (c d) m"))
            c_bias = consts.tile([1, 14], f32)
            nc.sync.dma_start(out=c_bias, in_=bias_d[:, :])
            c_bpat = consts.tile([1, 7], f32)
            nc.sync.dma_start(out=c_bpat, in_=bpat_d[:, :])
            c_ident = consts.tile([14, 14], f32)
            nc.sync.dma_start(out=c_ident, in_=ident_d[:, :])
            c_lmat = consts.tile([7, 7], f32)
            nc.sync.dma_start(out=c_lmat, in_=lmat_d[:, :])
            c_at = consts.tile([7, 512], f32r)
            nc.sync.dma_start(out=c_at, in_=at_d[:, :])
            c_atr = consts.tile([7, 4, 128], f32r)
            nc.sync.dma_start(out=c_atr, in_=atr_d.rearrange("t j p -> j t p"))

            Et_ps = pet.tile([7, 56], f32, tag="Et_ps")
            mT_ps = pmt.tile([7, 56], f32, tag="mT_ps")
            mT = mtp.tile([7, 56], f32r, tag="mT")

            def load_chunk(b, ci):
                # ---- load + phase 1: column sum for one (b, ci) tile ----
                xt = xpool.tile([128, 4, 512], f32r, tag="xt")
                eng = nc.sync if ci % 2 == 0 else nc.scalar
                eng.dma_start(
                    out=xt,
                    in_=xs[b, ci].rearrange("(p t) w -> p t w", t=4),
                )
                st = pstats.tile([1, 512], f32, tag="st")
                for t in range(4):
                    nc.tensor.matmul(st, c_ones, xt[:, t, :],
                                     start=(t == 0), stop=(t == 3))
                return st

            def load_finish(b, S, sts):
                # boundary rows 0..3 and 508..511 of all 4 channels
                nc.scalar.dma_start(
                    out=S[1:5, :, :],
                    in_=xs[b, :, 0:4, :].rearrange("c r w -> r c w"),
                )
                nc.scalar.dma_start(
                    out=S[5:9, :, :],
                    in_=xs[b, :, 508:512, :].rearrange("c r w -> r c w"),
                )
                for ci in range(CIN):
                    nc.vector.tensor_copy(S[0:1, ci, :], sts[ci])

            def stage_v(b, S, ctx):
                # ---- phase 2a: per-row summaries V = [T | edges], batched ----
                # V column groups, one per w-shift dw (7 cols each):
                #  dw=0: [T-S511, 0,  S0, S1, S508, S509, S510]
                #  dw=1: [T,      S0, S1, S2, S509, S510, S511]
                #  dw=2: [T-S0,   S1, S2, S3, S510, S511, 0   ]
                V = vpool.tile([9, CIN, 21], f32, tag="V")
                nc.vector.reduce_sum(V[:, :, 7:8], S, axis=mybir.AxisListType.X)
                edges = bass.AP(           # S columns {0,1,2, 509,510,511}
                    tensor=S.tensor, offset=S.offset,
                    ap=[S.ap[0], S.ap[1], [509, 2], [1, 3]],
                )
                nc.vector.tensor_copy(
                    V[:, :, 8:14].rearrange("q c (g e) -> q c g e", g=2), edges)
                nc.vector.tensor_sub(V[:, :, 0:1], V[:, :, 7:8], V[:, :, 13:14])
                nc.gpsimd.memset(V[:, :, 1:2], 0.0)
                nc.gpsimd.tensor_copy(V[:, :, 2:4], V[:, :, 8:10])
                nc.vector.tensor_copy(V[:, :, 4:7], S[:, :, 508:511])
                nc.vector.tensor_sub(V[:, :, 14:15], V[:, :, 7:8], V[:, :, 8:9])
                nc.vector.tensor_copy(V[:, :, 15:18], S[:, :, 1:4])
                nc.gpsimd.tensor_copy(V[:, :, 18:20], V[:, :, 12:14])
                nc.gpsimd.memset(V[:, :, 20:21], 0.0)
                ctx["V"] = V

            def stage_r(b, ctx):
                # ---- phase 2b: R summaries [14, 7] via tiny fp32 matmuls ----
                V = ctx["V"]
                Rb = pr.tile([14, 7], f32, tag="R")
                nc.tensor.matmul(Rb, c_bias, c_bpat, start=True, stop=False)
                for ci in range(CIN):
                    for dw in range(3):
                        last = (ci == CIN - 1 and dw == 2)
                        nc.tensor.matmul(
                            Rb, c_blk[:, ci * 3 + dw, :],
                            V[:, ci, 7 * dw:7 * dw + 7],
                            start=False, stop=last)
                ctx["Rb"] = Rb

            def stage_t(b, ctx):
                # ---- phase 3a: transpose ----
                Ep = small.tile([14, 7], f32, tag="Ep")
                nc.vector.tensor_copy(Ep, ctx["Rb"])
                nc.tensor.transpose(Et_ps[:, 14 * b:14 * b + 14], Ep, c_ident)

            def stage_m(b, ctx):
                # ---- phase 3b: m^T columns for this b ----
                Etb = small.tile([7, 14], f32, tag="Etb")
                nc.vector.tensor_copy(Etb, Et_ps[:, 14 * b:14 * b + 14])
                nc.tensor.matmul(mT_ps[:, 14 * b:14 * b + 14], c_lmat, Etb,
                                 start=True, stop=True)
                nc.vector.tensor_copy(mT[:, 14 * b:14 * b + 14],
                                      mT_ps[:, 14 * b:14 * b + 14])

            def stage_out(b, co):
                # ---- phase 4: upsample out = A @ m @ A^T, store ----
                g = b * CO + co
                tg_ps = ptg.tile([7, 512], f32, tag="tg_ps")
                nc.tensor.matmul(tg_ps, mT[:, g * 7:(g + 1) * 7], c_at,
                                 start=True, stop=True)
                tg = tgpool.tile([7, 512], f32r, tag="tg")
                nc.vector.tensor_copy(tg, tg_ps)
                ob = obuf.tile([128, 4, 512], f32, tag="ob")
                for t in range(4):
                    oc_ps = poc.tile([128, 512], f32, tag="oc")
                    nc.tensor.matmul(oc_ps, c_atr[:, t, :], tg,
                                     start=True, stop=True)
                    if t % 2 == 0:
                        nc.vector.tensor_copy(ob[:, t, :], oc_ps)
                    else:
                        nc.scalar.copy(ob[:, t, :], oc_ps)
                oeng = nc.sync if co == 0 else nc.scalar
                oeng.dma_start(
                    out=y[b, co].rearrange("(p t) w -> p t w", t=4),
                    in_=ob,
                )

            # Software-pipelined emission.  The engine streams are in-order,
            # so batch b's tail stages (each a short PE burst gated on a DVE
            # result) are interleaved between batch b+1's column-sum bursts:
            # every cross-engine latency hides behind ~2 us of streaming work.
            queue = []
            for b in range(BLOC):
                S = spool.tile([9, CIN, 512], f32r, tag="S")
                sts = []
                for ci in range(CIN):
                    sts.append(load_chunk(b, ci))
                    if queue:
                        queue.pop(0)()
                load_finish(b, S, sts)
                ctx = {}
                stage_v(b, S, ctx)
                queue += [
                    (lambda b=b, ctx=ctx: stage_r(b, ctx)),
                    (lambda b=b, ctx=ctx: stage_t(b, ctx)),
                    (lambda b=b, ctx=ctx: stage_m(b, ctx)),
                    (lambda b=b: stage_out(b, 0)),
                    (lambda b=b: stage_out(b, 1)),
                ]
                queue.pop(0)()
            for fn in queue:
                fn()
    return nc


def _get_prog():
    global _PROG
    if _PROG is None:
        _PROG = build_program()
        _PROG.finalize()
    return _PROG


# ---------------------------------------------------------------------------
# host entry point
# ---------------------------------------------------------------------------

def kernel(x, g_w, g_b, theta_w, theta_b):
    global LAST_EXEC_NS, LAST_TRACE_PATH
    from concourse.bass_utils import run_bass_kernel_spmd

    x = np.ascontiguousarray(np.asarray(x, dtype=np.float32))
    g_w = np.asarray(g_w, dtype=np.float32)
    g_b = np.asarray(g_b, dtype=np.float32)
    theta_w = np.asarray(theta_w, dtype=np.float32)
    theta_b = np.asarray(theta_b, dtype=np.float32)

    consts = build_consts(g_w, g_b, theta_w, theta_b)
    nc = _get_prog()
    in_maps = [
        {"xs": np.ascontiguousarray(x[c * BLOC:(c + 1) * BLOC]), **consts}
        for c in range(NCORES)
    ]
    res = run_bass_kernel_spmd(nc, in_maps, core_ids=list(range(NCORES)),
                               trace=TRACE)
    LAST_EXEC_NS = res.exec_time_ns
    if TRACE and res.instructions_and_trace is not None:
        LAST_TRACE_PATH = res.instructions_and_trace[1]
    return np.concatenate([res.results[c]["y"] for c in range(NCORES)], axis=0)
